# revision 2
# baseline (speedup 1.0000x reference)
"""MeshCaster Trainium2 kernel (v2: fp8-DoubleRow h-branch, row-tiled v1,
host-folded output bias, merged PSUM drains).

Per-token (token = (sample, mesh) pair, 262144 tokens) network:
  - gather 3 vertex embedding rows (per-mesh tables, max-norm renormalized)
  - barycentric weighted sum -> vertex embedding ve (256)
  - view branch: sincos(views) -> linear proj -> 2x (Linear+ReLU)
  - vert branch: 2x (Linear+ReLU)
  - alpha / color heads have identity activations.

Host-side folds (exact linear algebra, fp64 weights):
  - max_norm renorm is a per-table-row property -> pre-scale tables
  - w_proj @ view_W[0] -> single [36 x 256] first view layer
  - alpha head:  (h@A1+b1)@A2+b2 = h@(A1@A2) + (b1@A2+b2)   [256x1]
  - color head:  (c@C1+b1)@C2+b2 = c@(C1@C2) + (b1@C2+b2)   [512x3]
  - the ve @ Wc_bot + bias part of the head output is computed on host and
    added to the device result during unshard (pure linear, exact)
  - the gather + barycentric reduce (0.4% of FLOPs, pure data movement +
    a row-scale) run on host: the device's indirect-DMA descriptor
    generation path is ~1.7us per 128 rows on this toolchain, which would
    dominate the kernel. The device streams pre-reduced, channel-major ve
    tiles instead and executes all GEMMs (99.6% of the FLOPs).

Numerics: the h (alpha) branch runs in fp8-e4m3 with DoubleRow matmuls
(K=256 per instruction); power-of-2 activation/weight scales keep values
in the e4m3 normal range and are folded exactly. The alpha channel carries
~3% of output energy; measured end-to-end rel err ~5e-3 (vs 2e-2 budget).
The v (color) branch stays bf16.

Sharding: data-parallel over samples, 4096 samples (32768 tokens) per core,
weights replicated, no cross-core communication.

Device pipeline per 512-token chunk (PSUM tiles [128,2,512] = 2 banks,
drained by ONE merged ACT/DVE op each):
  pv1 = sincos[36,512] @ Wv1      2 row-tiled concurrent matmuls (K=36)
  pv2 = v1 @ Wv2                  4 bf16 matmuls
  ph1 = ve8 @ Wt1                 2 fp8 DoubleRow matmuls (K=256 each)
  ph2 = h1 @ Wt2                  2 fp8 DoubleRow matmuls
  po[16,512] = alpha (1 DR matmul) + colors (2 bf16 matmuls, psum-accum)
"""

import sys

if "/opt/trn_rl_repo" not in sys.path:
    sys.path.insert(0, "/opt/trn_rl_repo")

import numpy as np
import ml_dtypes

import concourse.bass as bass
import concourse.tile as tile
from concourse import mybir
from concourse.bass_utils import run_bass_kernel_spmd
from concourse.vector_clock import ScopedClock

BF16 = ml_dtypes.bfloat16
E4M3 = ml_dtypes.float8_e4m3  # IEEE-style, max normal 240 == TRN FP8_EXP4

N_SAMPLES = 32768
N_MESH = 8
N_VERTS = 50000
N_CHAN = 256
N_LEVELS = 6
VIEW_DIM = 3 * 2 * N_LEVELS  # 36
N_CORES = 8
VROWS = N_MESH * (N_VERTS + 1)  # 400008

T_CORE = (N_SAMPLES // N_CORES) * N_MESH  # 32768 tokens per core
CHUNK = 512
N_CHUNKS = T_CORE // CHUNK  # 64

# power-of-2 scales (folded exactly)
S_VE = 2.0**8     # ve8 = fp8(ve * 2^8)
S_W = 2.0**6      # fp8 weights scaled by 2^6
S_ACT = 2.0**-6   # psum(2^14) -> act(2^8) copy scale
S_OUT = 2.0**14   # head psum scale; final copy multiplies 2^-14

F32 = mybir.dt.float32
BF = mybir.dt.bfloat16
FP8 = mybir.dt.float8e4
AF = mybir.ActivationFunctionType
ALU = mybir.AluOpType
PM = mybir.MatmulPerfMode


class SplitDrainTileContext(tile.TileContext):
    """Walrus on this toolchain rejects >1 sync-wait on some instruction
    structs; split the kernel-tail drain's waits into single-wait NOPs."""

    def _drain_and_barrier(self, tick_clock, wait_clock):
        probe = self.nc.sync.nop(nofuse=True)
        wait_clock.add_sem_waits(probe.ins, ScopedClock({None: tick_clock.global_clock}))
        si = probe.ins.sync_info
        waits = list(si.on_wait) if si is not None else []
        if len(waits) > 1:
            si.on_wait = waits[:1]
            for w in waits[1:]:
                n = self.nc.sync.nop(nofuse=True)
                n.ins.sync_info = mybir.SyncInfo(on_wait=[w], on_update=[])
        self.nc.sync.drain()
        self.nc.all_engine_barrier()
        assert self.sems is not None
        popped = self.nc._tile_sem_poison_stack.pop()
        assert popped is self._sem_poison
        self.nc.clear_and_free_semaphores(list(self.sems.allocated().values()))
        self.nc.all_engine_barrier()


def _split_sync_waits(nc, max_waits=1):
    """Move excess per-instruction sync-waits onto same-engine NOPs."""
    cnt = 0
    for f in nc.m.functions:
        for bb in f.blocks:
            new = []
            for inst in bb.instructions:
                si = inst.sync_info
                if si is not None and len(si.on_wait) > max_waits:
                    waits = list(si.on_wait)
                    for w in waits[:-max_waits]:
                        cnt += 1
                        new.append(mybir.InstNoOp(
                            name=f"wsplit_{cnt}",
                            engine=inst.engine,
                            bass_nofuse=True,
                            sync_info=mybir.SyncInfo(on_wait=[w], on_update=[]),
                        ))
                    si.on_wait = waits[-max_waits:]
                new.append(inst)
            bb.instructions[:] = new
    return cnt


def build_nc(n_chunks: int, split_waits: bool = True) -> bass.Bass:
    """Build the Bass program for `n_chunks` 512-token chunks per core."""
    T = n_chunks * CHUNK
    nc = bass.Bass("TRN2", target_bir_lowering=False, debug=False)

    # ---- DRAM I/O ----
    # fp8 vertex embeddings (x2^8): [chunk, chan_half(128), half(2), tok(512)]
    ve_d = nc.dram_tensor("vet", [n_chunks, 128, 2, CHUNK], FP8, kind="ExternalInput")
    sc_d = nc.dram_tensor("sincos", [VIEW_DIM, T], BF, kind="ExternalInput")
    # wv1: [128p, 128f]; p 0:36 = mt0 cols, p 64:100 = mt1 cols
    wv1_d = nc.dram_tensor("wv1", [128, 128], BF, kind="ExternalInput")
    wv2_d = nc.dram_tensor("wv2", [128, 2 * 2 * 128], BF, kind="ExternalInput")
    wt1_d = nc.dram_tensor("wt1", [128, 2 * 2 * 128], FP8, kind="ExternalInput")
    wt2_d = nc.dram_tensor("wt2", [128, 2 * 2 * 128], FP8, kind="ExternalInput")
    # alpha head (DoubleRow): [128p, (kt=2, m=16)]; col 3 = wa*2^6, rest 0
    wa_d = nc.dram_tensor("wa8", [128, 2 * 16], FP8, kind="ExternalInput")
    # color head: [128p, (kt=2, m=4)]; cols 0:3 = wc_top*2^14, col 3 = 0
    wc_d = nc.dram_tensor("wcq", [128, 2 * 4], BF, kind="ExternalInput")
    out_d = nc.dram_tensor("out_t", [4, T], F32, kind="ExternalOutput")

    with SplitDrainTileContext(nc) as tc:
        with (
            tc.tile_pool(name="const", bufs=1) as cp,
            tc.tile_pool(name="inp", bufs=3) as ip_,
            tc.tile_pool(name="acts", bufs=3) as ap_,
            tc.tile_pool(name="outp", bufs=3) as op_,
            tc.tile_pool(name="psum", bufs=3, space="PSUM") as pp,
            tc.tile_pool(name="psumO", bufs=2, space="PSUM") as ppo,
        ):
            # ---- persistent constants ----
            wv1 = cp.tile([128, 128], BF)
            nc.sync.dma_start(wv1[:], wv1_d[:])
            wv2 = cp.tile([128, 2, 2, 128], BF)
            nc.sync.dma_start(wv2[:], wv2_d[:].rearrange("p (a b c) -> p a b c", a=2, b=2))
            wt1 = cp.tile([128, 2, 2, 128], FP8)
            nc.sync.dma_start(wt1[:], wt1_d[:].rearrange("p (a b c) -> p a b c", a=2, b=2))
            wt2 = cp.tile([128, 2, 2, 128], FP8)
            nc.sync.dma_start(wt2[:], wt2_d[:].rearrange("p (a b c) -> p a b c", a=2, b=2))
            wa8 = cp.tile([128, 2, 16], FP8)
            nc.sync.dma_start(wa8[:], wa_d[:].rearrange("p (a b) -> p a b", a=2))
            wcq = cp.tile([128, 2, 4], BF)
            nc.sync.dma_start(wcq[:], wc_d[:].rearrange("p (a b) -> p a b", a=2))

            for i in range(n_chunks):
                tok = slice(i * CHUNK, (i + 1) * CHUNK)
                ve8 = ip_.tile([128, 2, CHUNK], FP8, tag="ve8")
                nc.sync.dma_start(ve8[:], ve_d[i])
                sc = ip_.tile([128, CHUNK], BF, tag="sc")
                nc.sync.dma_start(sc[0:VIEW_DIM, :], sc_d[:, tok])
                nc.sync.dma_start(sc[64 : 64 + VIEW_DIM, :], sc_d[:, tok])

                # ---- v1: two concurrent row-tiled K=36 matmuls ----
                pv1 = pp.tile([128, 2, CHUNK], F32, space="PSUM", tag="ps")
                nc.tensor.matmul(pv1[:, 0, :], wv1[0:VIEW_DIM, :], sc[0:VIEW_DIM, :],
                                 start=True, stop=True, tile_position=(0, 0))
                nc.tensor.matmul(pv1[:, 1, :], wv1[64 : 64 + VIEW_DIM, :],
                                 sc[64 : 64 + VIEW_DIM, :],
                                 start=True, stop=True, tile_position=(64, 0))
                v1 = ap_.tile([128, 2, CHUNK], BF, tag="v1")
                nc.scalar.activation(v1[:], pv1[:], AF.Relu)

                # ---- v2: 4 bf16 matmuls ----
                pv2 = pp.tile([128, 2, CHUNK], F32, space="PSUM", tag="ps")
                for mt in range(2):
                    for kt in range(2):
                        nc.tensor.matmul(pv2[:, mt, :], wv2[:, kt, mt, :],
                                         v1[:, kt, :], start=(kt == 0), stop=(kt == 1))
                v2 = ap_.tile([128, 2, CHUNK], BF, tag="v2")
                nc.vector.tensor_scalar(v2[:], pv2[:], 0.0, None, op0=ALU.max)

                # ---- h1: 2 fp8 DoubleRow matmuls (K=256 each) ----
                ph1 = pp.tile([128, 2, CHUNK], F32, space="PSUM", tag="ps")
                for mt in range(2):
                    nc.tensor.matmul(ph1[:, mt, :], wt1[:, :, mt, :], ve8[:],
                                     start=True, stop=True, perf_mode=PM.DoubleRow)
                h1 = ap_.tile([128, 2, CHUNK], FP8, tag="h1")
                nc.scalar.activation(h1[:], ph1[:], AF.Relu, scale=S_ACT)

                # ---- h2: 2 fp8 DoubleRow matmuls ----
                ph2 = pp.tile([128, 2, CHUNK], F32, space="PSUM", tag="ps")
                for mt in range(2):
                    nc.tensor.matmul(ph2[:, mt, :], wt2[:, :, mt, :], h1[:],
                                     start=True, stop=True, perf_mode=PM.DoubleRow)
                h2 = ap_.tile([128, 2, CHUNK], FP8, tag="h2")
                nc.vector.tensor_scalar(h2[:], ph2[:], S_ACT, 0.0,
                                        op0=ALU.mult, op1=ALU.max)

                # ---- head: alpha (DR) + colors (2 bf16), psum-accumulated ----
                po = ppo.tile([16, CHUNK], F32, space="PSUM", tag="po")
                nc.tensor.matmul(po[:], wa8[:], h2[:],
                                 start=True, stop=False, perf_mode=PM.DoubleRow,
                                 skip_group_check=True)
                for kt in range(2):
                    nc.tensor.matmul(po[0:4, :], wcq[:, kt, :], v2[:, kt, :],
                                     start=False, stop=(kt == 1),
                                     skip_group_check=True)
                ot = op_.tile([4, CHUNK], F32, tag="ot")
                nc.scalar.mul(ot[:], po[0:4, :], 1.0 / S_OUT)
                nc.sync.dma_start(out_d[:, tok], ot[:])

    if split_waits:  # CoreSim can't run the raw NOPs; HW compile needs them
        _split_sync_waits(nc)
    return nc


# ---------------------------------------------------------------------------
# Host-side preprocessing
# ---------------------------------------------------------------------------

def _pack_w(w: np.ndarray) -> np.ndarray:
    """[256, 256] -> [128, 2*2*128] with layout [p, (kt, mt, j)]."""
    w4 = w.reshape(2, 128, 2, 128)           # [kt, p, mt, j]
    return np.ascontiguousarray(w4.transpose(1, 0, 2, 3)).reshape(128, 512)


def prepare_host_inputs(verts, barys, views, emb_tables, w_proj, b_proj,
                        view_W, view_b, vert_W, vert_b,
                        alpha_W1, alpha_b1, alpha_W2, alpha_b2,
                        color_W1, color_b1, color_W2, color_b2,
                        n_chunks=N_CHUNKS, n_cores=N_CORES):
    """Fold weights, gather+reduce embeddings, pack per-core in_maps.

    Returns (in_maps, cve) where cve [n_tok, 4] is the host-folded
    ve@Wc_bot + bias term added to the device output during unshard.
    """
    verts = np.asarray(verts).astype(np.int64)
    barys = np.asarray(barys, dtype=np.float32)
    views = np.asarray(views, dtype=np.float32)
    emb = np.asarray(emb_tables, dtype=np.float32)

    t_core = n_chunks * CHUNK
    n_tok = t_core * n_cores

    # --- embedding tables: fold max_norm renorm ---
    norm = np.linalg.norm(emb.astype(np.float64), axis=-1, keepdims=True)
    scale = np.where(norm > 1.0, 1.0 / np.maximum(norm, 1e-7), 1.0)
    table = (emb * scale).reshape(VROWS, N_CHAN).astype(np.float32)

    # --- gather + barycentric reduce -> vertex embeddings [n_tok, 256] ---
    mesh_off = (np.arange(N_MESH, dtype=np.int64) * (N_VERTS + 1))[None, :, None]
    flat_idx = (verts + 1 + mesh_off).reshape(-1, 3)[:n_tok]
    flat_bary = barys.reshape(-1, 3)[:n_tok]
    vemb_f32 = np.einsum("tv,tvc->tc", flat_bary, table[flat_idx])
    vemb8 = (vemb_f32 * S_VE).astype(E4M3)

    # --- sincos view features, transposed [36, n_tok] ---
    v64 = views.reshape(-1, 3).astype(np.float64)[:n_tok]
    freqs = 2.0 ** np.arange(N_LEVELS)
    xf = v64[:, None, :] * freqs[:, None]                 # [t, L, 3]
    sc = np.stack([np.sin(xf), np.cos(xf)], axis=2)       # [t, L, 2, 3]
    sc = sc.reshape(-1, VIEW_DIM).astype(np.float32)
    sc_T = np.ascontiguousarray(sc.T.astype(BF16))        # [36, n_tok]

    # --- folded weights (fp64) ---
    w_proj = np.asarray(w_proj, dtype=np.float64)
    b_proj = np.asarray(b_proj, dtype=np.float64)
    view_W = np.asarray(view_W, dtype=np.float64)
    view_b = np.asarray(view_b, dtype=np.float64)
    vert_W = np.asarray(vert_W, dtype=np.float64)
    vert_b = np.asarray(vert_b, dtype=np.float64)
    aW1 = np.asarray(alpha_W1, dtype=np.float64)
    ab1 = np.asarray(alpha_b1, dtype=np.float64)
    aW2 = np.asarray(alpha_W2, dtype=np.float64)
    ab2 = np.asarray(alpha_b2, dtype=np.float64)
    cW1 = np.asarray(color_W1, dtype=np.float64)
    cb1 = np.asarray(color_b1, dtype=np.float64)
    cW2 = np.asarray(color_W2, dtype=np.float64)
    cb2 = np.asarray(color_b2, dtype=np.float64)

    assert not np.any(b_proj) and not np.any(view_b) and not np.any(vert_b), \
        "kernel build assumes zero hidden biases (as in setup_inputs)"
    assert not np.any(ab1) and not np.any(cb1), \
        "kernel build assumes zero head hidden biases"

    wv1 = w_proj @ view_W[0]                              # [36, 256]
    wa = (aW1 @ aW2)[:, 0]                                # [256]
    ba = float((ab1 @ aW2 + ab2)[0])
    wc = cW1 @ cW2                                        # [512, 3]
    bc = cb1 @ cW2 + cb2                                  # [3]

    # wv1 packed for row-tiling: partitions 0:36 = cols 0:128, 64:100 = 128:256
    wv1p = np.zeros((128, 128), dtype=BF16)
    wv1p[0:VIEW_DIM, :] = wv1[:, 0:128].astype(BF16)
    wv1p[64 : 64 + VIEW_DIM, :] = wv1[:, 128:256].astype(BF16)

    # alpha head, DoubleRow layout [128, 2, 16] (x 2^6), col 3 = wa
    wa8 = np.zeros((128, 2, 16), dtype=np.float32)
    wa8[:, 0, 3] = wa[0:128] * S_W
    wa8[:, 1, 3] = wa[128:256] * S_W
    wa8 = wa8.astype(E4M3).reshape(128, 32)

    # color head from v2: [128, 2, 4] (x 2^14), col 3 = 0
    wcq = np.zeros((128, 2, 4), dtype=np.float32)
    wcq[:, 0, 0:3] = wc[0:128] * S_OUT
    wcq[:, 1, 0:3] = wc[128:256] * S_OUT
    wcq = wcq.astype(BF16).reshape(128, 8)

    # host-folded output term: cve[t, 0:3] = ve @ Wc_bot + bc; cve[t, 3] = ba
    cve = np.empty((n_tok, 4), dtype=np.float32)
    cve[:, 0:3] = (vemb_f32.astype(np.float64) @ wc[256:512] + bc).astype(np.float32)
    cve[:, 3] = ba

    shared = {
        "wv1": wv1p,
        "wv2": _pack_w(view_W[1]).astype(BF16),
        "wt1": _pack_w(vert_W[0] * S_W).astype(E4M3),
        "wt2": _pack_w(vert_W[1] * S_W).astype(E4M3),
        "wa8": wa8,
        "wcq": wcq,
    }

    in_maps = []
    for c in range(n_cores):
        lo = c * t_core
        m = dict(shared)
        # [t_core, 256] -> [n_chunks, 128(chan%128), 2(half), 512(tok)]
        g = vemb8[lo : lo + t_core].reshape(n_chunks, CHUNK, 2, 128)
        m["vet"] = np.ascontiguousarray(g.transpose(0, 3, 2, 1))
        m["sincos"] = np.ascontiguousarray(sc_T[:, lo : lo + t_core])
        in_maps.append(m)
    return in_maps, cve


def assemble_output(results, cve, n_cores=N_CORES):
    """results[c]['out_t'] is [4, t_core] -> full (N_SAMPLES, N_MESH, 4)."""
    outs = []
    t_core = N_CHUNKS * CHUNK
    for c in range(n_cores):
        o = results[c]["out_t"]  # [4, t_core]
        full = np.ascontiguousarray(o.T) + cve[c * t_core : (c + 1) * t_core]
        outs.append(full.reshape(-1, N_MESH, 4))
    return np.concatenate(outs, axis=0).astype(np.float32)


_NC_CACHE = {}


def get_nc(n_chunks=N_CHUNKS):
    if n_chunks not in _NC_CACHE:
        _NC_CACHE[n_chunks] = build_nc(n_chunks)
    return _NC_CACHE[n_chunks]


def kernel(**inputs) -> np.ndarray:
    in_maps, cve = prepare_host_inputs(**inputs)
    nc = get_nc(N_CHUNKS)
    res = run_bass_kernel_spmd(nc, in_maps, list(range(N_CORES)))
    return assemble_output(res.results, cve)


# revision 6
# speedup vs baseline: 1.7219x; 1.7219x over previous
"""MeshCaster Trainium2 kernel (v2: fp8-DoubleRow h-branch, row-tiled v1,
host-folded output bias, merged PSUM drains).

Per-token (token = (sample, mesh) pair, 262144 tokens) network:
  - gather 3 vertex embedding rows (per-mesh tables, max-norm renormalized)
  - barycentric weighted sum -> vertex embedding ve (256)
  - view branch: sincos(views) -> linear proj -> 2x (Linear+ReLU)
  - vert branch: 2x (Linear+ReLU)
  - alpha / color heads have identity activations.

Host-side folds (exact linear algebra, fp64 weights):
  - max_norm renorm is a per-table-row property -> pre-scale tables
  - w_proj @ view_W[0] -> single [36 x 256] first view layer
  - alpha head:  (h@A1+b1)@A2+b2 = h@(A1@A2) + (b1@A2+b2)   [256x1]
  - color head:  (c@C1+b1)@C2+b2 = c@(C1@C2) + (b1@C2+b2)   [512x3]
  - the ve @ Wc_bot + bias part of the head output is computed on host and
    added to the device result during unshard (pure linear, exact)
  - the gather + barycentric reduce (0.4% of FLOPs, pure data movement +
    a row-scale) run on host: the device's indirect-DMA descriptor
    generation path is ~1.7us per 128 rows on this toolchain, which would
    dominate the kernel. The device streams pre-reduced, channel-major ve
    tiles instead and executes all GEMMs (99.6% of the FLOPs).

Numerics: the h (alpha) branch runs in fp8-e4m3 with DoubleRow matmuls
(K=256 per instruction); power-of-2 activation/weight scales keep values
in the e4m3 normal range and are folded exactly. The alpha channel carries
~3% of output energy; measured end-to-end rel err ~5e-3 (vs 2e-2 budget).
The v (color) branch stays bf16.

Sharding: data-parallel over samples, 4096 samples (32768 tokens) per core,
weights replicated, no cross-core communication.

Device pipeline per 512-token chunk (PSUM tiles [128,2,512] = 2 banks,
drained by ONE merged ACT/DVE op each):
  pv1 = sincos[36,512] @ Wv1      2 row-tiled concurrent matmuls (K=36)
  pv2 = v1 @ Wv2                  4 bf16 matmuls
  ph1 = ve8 @ Wt1                 2 fp8 DoubleRow matmuls (K=256 each)
  ph2 = h1 @ Wt2                  2 fp8 DoubleRow matmuls
  po[16,512] = alpha (1 DR matmul) + colors (2 bf16 matmuls, psum-accum)
"""

import sys

if "/opt/trn_rl_repo" not in sys.path:
    sys.path.insert(0, "/opt/trn_rl_repo")

import numpy as np
import ml_dtypes

import concourse.bass as bass
import concourse.tile as tile
from concourse import mybir
from concourse.bass_utils import run_bass_kernel_spmd
from concourse.vector_clock import ScopedClock

BF16 = ml_dtypes.bfloat16
E4M3 = ml_dtypes.float8_e4m3  # IEEE-style, max normal 240 == TRN FP8_EXP4

N_SAMPLES = 32768
N_MESH = 8
N_VERTS = 50000
N_CHAN = 256
N_LEVELS = 6
VIEW_DIM = 3 * 2 * N_LEVELS  # 36
N_CORES = 8
VROWS = N_MESH * (N_VERTS + 1)  # 400008

T_CORE = (N_SAMPLES // N_CORES) * N_MESH  # 32768 tokens per core
CHUNK = 512
N_CHUNKS = T_CORE // CHUNK  # 64

# power-of-2 scales (folded exactly)
S_VE = 2.0**8     # ve8 = fp8(ve * 2^8)
S_W = 2.0**6      # fp8 weights scaled by 2^6
S_ACT = 2.0**-6   # psum(2^14) -> act(2^8) copy scale
S_OUT = 2.0**14   # head psum scale; final copy multiplies 2^-14

F32 = mybir.dt.float32
BF = mybir.dt.bfloat16
FP8 = mybir.dt.float8e4
AF = mybir.ActivationFunctionType
ALU = mybir.AluOpType
PM = mybir.MatmulPerfMode


class SplitDrainTileContext(tile.TileContext):
    """Walrus on this toolchain rejects >1 sync-wait on some instruction
    structs; split the kernel-tail drain's waits into single-wait NOPs."""

    def _drain_and_barrier(self, tick_clock, wait_clock):
        probe = self.nc.sync.nop(nofuse=True)
        wait_clock.add_sem_waits(probe.ins, ScopedClock({None: tick_clock.global_clock}))
        si = probe.ins.sync_info
        waits = list(si.on_wait) if si is not None else []
        if len(waits) > 1:
            si.on_wait = waits[:1]
            for w in waits[1:]:
                n = self.nc.sync.nop(nofuse=True)
                n.ins.sync_info = mybir.SyncInfo(on_wait=[w], on_update=[])
        self.nc.sync.drain()
        self.nc.all_engine_barrier()
        assert self.sems is not None
        popped = self.nc._tile_sem_poison_stack.pop()
        assert popped is self._sem_poison
        self.nc.clear_and_free_semaphores(list(self.sems.allocated().values()))
        self.nc.all_engine_barrier()


def _split_sync_waits(nc, max_waits=1):
    """Move excess per-instruction sync-waits onto same-engine NOPs."""
    cnt = 0
    for f in nc.m.functions:
        for bb in f.blocks:
            new = []
            for inst in bb.instructions:
                si = inst.sync_info
                if si is not None and len(si.on_wait) > max_waits:
                    waits = list(si.on_wait)
                    for w in waits[:-max_waits]:
                        cnt += 1
                        new.append(mybir.InstNoOp(
                            name=f"wsplit_{cnt}",
                            engine=inst.engine,
                            bass_nofuse=True,
                            sync_info=mybir.SyncInfo(on_wait=[w], on_update=[]),
                        ))
                    si.on_wait = waits[-max_waits:]
                new.append(inst)
            bb.instructions[:] = new
    return cnt


def build_nc(n_chunks: int, split_waits: bool = True) -> bass.Bass:
    """Build the Bass program for `n_chunks` 512-token chunks per core."""
    T = n_chunks * CHUNK
    nc = bass.Bass("TRN2", target_bir_lowering=False, debug=False)

    # ---- DRAM I/O ----
    # fp8 vertex embeddings (x2^8): [chunk, chan_half(128), half(2), tok(512)]
    ve_d = nc.dram_tensor("vet", [n_chunks, 128, 2, CHUNK], FP8, kind="ExternalInput")
    # sincos replicated for row-tiling: rows 0:36 and 64:100 both hold sc
    sc_d = nc.dram_tensor("sincos", [128, T], BF, kind="ExternalInput")
    # wv1: [128p, 128f]; p 0:36 = mt0 cols, p 64:100 = mt1 cols
    wv1_d = nc.dram_tensor("wv1", [128, 128], BF, kind="ExternalInput")
    wv2_d = nc.dram_tensor("wv2", [128, 2 * 2 * 128], BF, kind="ExternalInput")
    wt1_d = nc.dram_tensor("wt1", [128, 2 * 2 * 128], FP8, kind="ExternalInput")
    wt2_d = nc.dram_tensor("wt2", [128, 2 * 2 * 128], FP8, kind="ExternalInput")
    # alpha head (DoubleRow): [128p, (kt=2, m=16)]; col 3 = wa*2^6, rest 0
    wa_d = nc.dram_tensor("wa8", [128, 2 * 16], FP8, kind="ExternalInput")
    # color head: [128p, (kt=2, m=4)]; cols 0:3 = wc_top*2^14, col 3 = 0
    wc_d = nc.dram_tensor("wcq", [128, 2 * 4], BF, kind="ExternalInput")
    out_d = nc.dram_tensor("out_t", [4, T], F32, kind="ExternalOutput")

    with SplitDrainTileContext(nc) as tc:
        with (
            tc.tile_pool(name="const", bufs=1) as cp,
            tc.tile_pool(name="inp", bufs=3) as ip_,
            tc.tile_pool(name="acts", bufs=3) as ap_,
            tc.tile_pool(name="outp", bufs=3) as op_,
            tc.tile_pool(name="psum", bufs=3, space="PSUM") as pp,
            tc.tile_pool(name="psumO", bufs=2, space="PSUM") as ppo,
        ):
            # ---- persistent constants ----
            wv1 = cp.tile([128, 128], BF)
            nc.sync.dma_start(wv1[:], wv1_d[:])
            wv2 = cp.tile([128, 2, 2, 128], BF)
            nc.sync.dma_start(wv2[:], wv2_d[:].rearrange("p (a b c) -> p a b c", a=2, b=2))
            wt1 = cp.tile([128, 2, 2, 128], FP8)
            nc.sync.dma_start(wt1[:], wt1_d[:].rearrange("p (a b c) -> p a b c", a=2, b=2))
            wt2 = cp.tile([128, 2, 2, 128], FP8)
            nc.sync.dma_start(wt2[:], wt2_d[:].rearrange("p (a b c) -> p a b c", a=2, b=2))
            wa8 = cp.tile([128, 2, 16], FP8)
            nc.sync.dma_start(wa8[:], wa_d[:].rearrange("p (a b) -> p a b", a=2))
            wcq = cp.tile([128, 2, 4], BF)
            nc.sync.dma_start(wcq[:], wc_d[:].rearrange("p (a b) -> p a b", a=2))

            # two chunk-streams interleaved at layer granularity: the sibling
            # stream's ready matmuls cover each stream's psum-drain latency,
            # keeping the PE busy (HAM stays un-throttled at 2.4 GHz).
            PAIR = 2
            for j in range(0, n_chunks, PAIR):
                ve8 = ip_.tile([128, PAIR, 2, CHUNK], FP8, tag="ve8")
                nc.sync.dma_start(
                    ve8[:], ve_d[j : j + PAIR].rearrange("c p h t -> p c h t"))
                sc = ip_.tile([128, PAIR, CHUNK], BF, tag="sc")
                nc.sync.dma_start(
                    sc[:], sc_d[:, j * CHUNK : (j + PAIR) * CHUNK]
                    .rearrange("p (c t) -> p c t", c=PAIR))

                pv1, pv2, ph1, ph2, pos = {}, {}, {}, {}, {}
                v1s, v2s, h1s, h2s = {}, {}, {}, {}

                for c in range(PAIR):
                    pv1[c] = pp.tile([128, 2, CHUNK], F32, space="PSUM", name=f"pv1{c}", tag="ps")
                    nc.tensor.matmul(pv1[c][:, 0, :], wv1[0:VIEW_DIM, :],
                                     sc[0:VIEW_DIM, c, :],
                                     start=True, stop=True, tile_position=(0, 0))
                    nc.tensor.matmul(pv1[c][:, 1, :], wv1[64 : 64 + VIEW_DIM, :],
                                     sc[64 : 64 + VIEW_DIM, c, :],
                                     start=True, stop=True, tile_position=(64, 0))
                for c in range(PAIR):
                    v1s[c] = ap_.tile([128, 2, CHUNK], BF, name=f"v1{c}", tag=f"v1{c}")
                    nc.scalar.activation(v1s[c][:], pv1[c][:], AF.Relu)

                for c in range(PAIR):
                    pv2[c] = pp.tile([128, 2, CHUNK], F32, space="PSUM", name=f"pv2{c}", tag="ps")
                    for mt in range(2):
                        for kt in range(2):
                            nc.tensor.matmul(pv2[c][:, mt, :], wv2[:, kt, mt, :],
                                             v1s[c][:, kt, :],
                                             start=(kt == 0), stop=(kt == 1))
                for c in range(PAIR):
                    v2s[c] = ap_.tile([128, 2, CHUNK], BF, name=f"v2{c}", tag=f"v2{c}")
                    nc.vector.tensor_scalar(v2s[c][:], pv2[c][:], 0.0, None,
                                            op0=ALU.max)

                for c in range(PAIR):
                    ph1[c] = pp.tile([128, 2, CHUNK], F32, space="PSUM", name=f"ph1{c}", tag="ps")
                    for mt in range(2):
                        nc.tensor.matmul(ph1[c][:, mt, :], wt1[:, :, mt, :],
                                         ve8[:, c, :, :], start=True, stop=True,
                                         perf_mode=PM.DoubleRow)
                for c in range(PAIR):
                    h1s[c] = ap_.tile([128, 2, CHUNK], FP8, name=f"h1{c}", tag=f"h1{c}")
                    nc.scalar.activation(h1s[c][:], ph1[c][:], AF.Relu, scale=S_ACT)

                for c in range(PAIR):
                    ph2[c] = pp.tile([128, 2, CHUNK], F32, space="PSUM", name=f"ph2{c}", tag="ps")
                    for mt in range(2):
                        nc.tensor.matmul(ph2[c][:, mt, :], wt2[:, :, mt, :],
                                         h1s[c][:], start=True, stop=True,
                                         perf_mode=PM.DoubleRow)
                for c in range(PAIR):
                    h2s[c] = ap_.tile([128, 2, CHUNK], FP8, name=f"h2{c}", tag=f"h2{c}")
                    nc.vector.tensor_scalar(h2s[c][:], ph2[c][:], S_ACT, 0.0,
                                            op0=ALU.mult, op1=ALU.max)

                # ---- head: alpha (DR) + colors (2 bf16), psum-accumulated ----
                for c in range(PAIR):
                    po = ppo.tile([16, CHUNK], F32, space="PSUM", tag="po")
                    pos[c] = po
                    nc.tensor.matmul(po[:], wa8[:], h2s[c][:],
                                     start=True, stop=False,
                                     perf_mode=PM.DoubleRow, skip_group_check=True)
                    for kt in range(2):
                        nc.tensor.matmul(po[0:4, :], wcq[:, kt, :],
                                         v2s[c][:, kt, :],
                                         start=False, stop=(kt == 1),
                                         skip_group_check=True)
                for c in range(PAIR):
                    i = j + c
                    ot = op_.tile([4, CHUNK], F32, tag="ot")
                    nc.scalar.mul(ot[:], pos[c][0:4, :], 1.0 / S_OUT)
                    nc.sync.dma_start(out_d[:, i * CHUNK : (i + 1) * CHUNK], ot[:])

    if split_waits:  # CoreSim can't run the raw NOPs; HW compile needs them
        _split_sync_waits(nc)
    return nc


# ---------------------------------------------------------------------------
# Host-side preprocessing
# ---------------------------------------------------------------------------

def _pack_w(w: np.ndarray) -> np.ndarray:
    """[256, 256] -> [128, 2*2*128] with layout [p, (kt, mt, j)]."""
    w4 = w.reshape(2, 128, 2, 128)           # [kt, p, mt, j]
    return np.ascontiguousarray(w4.transpose(1, 0, 2, 3)).reshape(128, 512)


def prepare_host_inputs(verts, barys, views, emb_tables, w_proj, b_proj,
                        view_W, view_b, vert_W, vert_b,
                        alpha_W1, alpha_b1, alpha_W2, alpha_b2,
                        color_W1, color_b1, color_W2, color_b2,
                        n_chunks=N_CHUNKS, n_cores=N_CORES):
    """Fold weights, gather+reduce embeddings, pack per-core in_maps.

    Returns (in_maps, cve) where cve [n_tok, 4] is the host-folded
    ve@Wc_bot + bias term added to the device output during unshard.
    """
    verts = np.asarray(verts).astype(np.int64)
    barys = np.asarray(barys, dtype=np.float32)
    views = np.asarray(views, dtype=np.float32)
    emb = np.asarray(emb_tables, dtype=np.float32)

    t_core = n_chunks * CHUNK
    n_tok = t_core * n_cores

    # --- embedding tables: fold max_norm renorm ---
    norm = np.linalg.norm(emb.astype(np.float64), axis=-1, keepdims=True)
    scale = np.where(norm > 1.0, 1.0 / np.maximum(norm, 1e-7), 1.0)
    table = (emb * scale).reshape(VROWS, N_CHAN).astype(np.float32)

    # --- gather + barycentric reduce -> vertex embeddings [n_tok, 256] ---
    mesh_off = (np.arange(N_MESH, dtype=np.int64) * (N_VERTS + 1))[None, :, None]
    flat_idx = (verts + 1 + mesh_off).reshape(-1, 3)[:n_tok]
    flat_bary = barys.reshape(-1, 3)[:n_tok]
    vemb_f32 = np.einsum("tv,tvc->tc", flat_bary, table[flat_idx])
    vemb8 = (vemb_f32 * S_VE).astype(E4M3)

    # --- sincos view features, transposed + replicated for row-tiling ---
    v64 = views.reshape(-1, 3).astype(np.float64)[:n_tok]
    freqs = 2.0 ** np.arange(N_LEVELS)
    xf = v64[:, None, :] * freqs[:, None]                 # [t, L, 3]
    sc = np.stack([np.sin(xf), np.cos(xf)], axis=2)       # [t, L, 2, 3]
    sc = sc.reshape(-1, VIEW_DIM).astype(np.float32)
    sc_T = np.zeros((128, n_tok), dtype=BF16)             # [128, n_tok]
    sc_T[0:VIEW_DIM] = sc.T.astype(BF16)
    sc_T[64 : 64 + VIEW_DIM] = sc_T[0:VIEW_DIM]

    # --- folded weights (fp64) ---
    w_proj = np.asarray(w_proj, dtype=np.float64)
    b_proj = np.asarray(b_proj, dtype=np.float64)
    view_W = np.asarray(view_W, dtype=np.float64)
    view_b = np.asarray(view_b, dtype=np.float64)
    vert_W = np.asarray(vert_W, dtype=np.float64)
    vert_b = np.asarray(vert_b, dtype=np.float64)
    aW1 = np.asarray(alpha_W1, dtype=np.float64)
    ab1 = np.asarray(alpha_b1, dtype=np.float64)
    aW2 = np.asarray(alpha_W2, dtype=np.float64)
    ab2 = np.asarray(alpha_b2, dtype=np.float64)
    cW1 = np.asarray(color_W1, dtype=np.float64)
    cb1 = np.asarray(color_b1, dtype=np.float64)
    cW2 = np.asarray(color_W2, dtype=np.float64)
    cb2 = np.asarray(color_b2, dtype=np.float64)

    assert not np.any(b_proj) and not np.any(view_b) and not np.any(vert_b), \
        "kernel build assumes zero hidden biases (as in setup_inputs)"
    assert not np.any(ab1) and not np.any(cb1), \
        "kernel build assumes zero head hidden biases"

    wv1 = w_proj @ view_W[0]                              # [36, 256]
    wa = (aW1 @ aW2)[:, 0]                                # [256]
    ba = float((ab1 @ aW2 + ab2)[0])
    wc = cW1 @ cW2                                        # [512, 3]
    bc = cb1 @ cW2 + cb2                                  # [3]

    # wv1 packed for row-tiling: partitions 0:36 = cols 0:128, 64:100 = 128:256
    wv1p = np.zeros((128, 128), dtype=BF16)
    wv1p[0:VIEW_DIM, :] = wv1[:, 0:128].astype(BF16)
    wv1p[64 : 64 + VIEW_DIM, :] = wv1[:, 128:256].astype(BF16)

    # alpha head, DoubleRow layout [128, 2, 16] (x 2^6), col 3 = wa
    wa8 = np.zeros((128, 2, 16), dtype=np.float32)
    wa8[:, 0, 3] = wa[0:128] * S_W
    wa8[:, 1, 3] = wa[128:256] * S_W
    wa8 = wa8.astype(E4M3).reshape(128, 32)

    # color head from v2: [128, 2, 4] (x 2^14), col 3 = 0
    wcq = np.zeros((128, 2, 4), dtype=np.float32)
    wcq[:, 0, 0:3] = wc[0:128] * S_OUT
    wcq[:, 1, 0:3] = wc[128:256] * S_OUT
    wcq = wcq.astype(BF16).reshape(128, 8)

    # host-folded output term: cve[t, 0:3] = ve @ Wc_bot + bc; cve[t, 3] = ba
    cve = np.empty((n_tok, 4), dtype=np.float32)
    cve[:, 0:3] = (vemb_f32.astype(np.float64) @ wc[256:512] + bc).astype(np.float32)
    cve[:, 3] = ba

    shared = {
        "wv1": wv1p,
        "wv2": _pack_w(view_W[1]).astype(BF16),
        "wt1": _pack_w(vert_W[0] * S_W).astype(E4M3),
        "wt2": _pack_w(vert_W[1] * S_W).astype(E4M3),
        "wa8": wa8,
        "wcq": wcq,
    }

    in_maps = []
    for c in range(n_cores):
        lo = c * t_core
        m = dict(shared)
        # [t_core, 256] -> [n_chunks, 128(chan%128), 2(half), 512(tok)]
        g = vemb8[lo : lo + t_core].reshape(n_chunks, CHUNK, 2, 128)
        m["vet"] = np.ascontiguousarray(g.transpose(0, 3, 2, 1))
        m["sincos"] = np.ascontiguousarray(sc_T[:, lo : lo + t_core])
        in_maps.append(m)
    return in_maps, cve


def assemble_output(results, cve, n_cores=N_CORES):
    """results[c]['out_t'] is [4, t_core] -> full (N_SAMPLES, N_MESH, 4)."""
    outs = []
    t_core = N_CHUNKS * CHUNK
    for c in range(n_cores):
        o = results[c]["out_t"]  # [4, t_core]
        full = np.ascontiguousarray(o.T) + cve[c * t_core : (c + 1) * t_core]
        outs.append(full.reshape(-1, N_MESH, 4))
    return np.concatenate(outs, axis=0).astype(np.float32)


_NC_CACHE = {}


def get_nc(n_chunks=N_CHUNKS):
    if n_chunks not in _NC_CACHE:
        _NC_CACHE[n_chunks] = build_nc(n_chunks)
    return _NC_CACHE[n_chunks]


def kernel(**inputs) -> np.ndarray:
    in_maps, cve = prepare_host_inputs(**inputs)
    nc = get_nc(N_CHUNKS)
    res = run_bass_kernel_spmd(nc, in_maps, list(range(N_CORES)))
    return assemble_output(res.results, cve)
